# revision 23
# baseline (speedup 1.0000x reference)
"""Trainium2 Bass kernel for nn_LinkPredictor.

Reference computation (B=4, N=256, T=16, F=128, H=256):
    h = mean_T(nodefeat)                      # [B,N,F]
    a = h @ W1[:, :F].T                       # [B,N,H]
    c = h @ W1[:, F:].T                       # [B,N,H]
    logits[b,i,j] = W2[0] . relu(a[b,i] + c[b,j] + b1) + b2   # [B,N,N]

Sharding: 8 cores; core k handles batch b=k//2, i-half k%2 (128 i-rows x
256 j-cols of one batch's NxN grid).  Each core only needs nodefeat[b].

v4 = baseline pairwise structure (tight act pool, in-loop drains, VE/SE
act split) + a lean prologue:
  - inputs on BOTH HWDGE queues (sync+scalar), arrival-ordered; 0.68MB
    less traffic than baseline (w2 diagonal matrix -> [128,2,63] sliding
    window; W1 in bf16; smat/b1/b2 packed into one tiny param).
  - wide dummy matmuls during the DMA wait pre-trip the HAM clock gate
    (PE at 2.4GHz when the pairwise phase starts instead of ~10us in).
  - early ScalarE activation-table load.
"""

import os
import sys

import numpy as np

_B, _N, _T, _F, _H = 4, 256, 16, 128, 256
_NCORES = 8

_CACHE = {}


def _ensure_paths():
    for p in (
        "/root/.axon_site",
        "/root/.axon_site/_ro/trn_rl_repo",
        "/root/.axon_site/_ro/pypackages",
        "/opt/trn_rl_repo",
    ):
        if os.path.isdir(p) and p not in sys.path:
            sys.path.append(p)


# wee param layout (f32 columns; scalar cols 16B-aligned)
_WEE_SMAT = 0   # [0:8) smat
_WEE_B1T0 = 8   # b1 (t=0)
_WEE_B1T1 = 12  # b1 (t=1)
_WEE_B2C = 16   # b2
_WEE_W = 20


def build_nc():
    """Build the per-core Bass program (same program for all 8 cores)."""
    _ensure_paths()
    import concourse.mybir as mybir
    import concourse.tile as tile
    from concourse import bacc

    f32 = mybir.dt.float32
    bf16 = mybir.dt.bfloat16
    Alu = mybir.AluOpType
    Act = mybir.ActivationFunctionType

    nc = bacc.Bacc("TRN2", target_bir_lowering=False, debug=False)

    nf = nc.declare_dram_parameter("nf", [128, 32, 128], bf16, isOutput=False)
    wee = nc.declare_dram_parameter("wee", [128, _WEE_W], f32, isOutput=False)
    w1ab = nc.declare_dram_parameter("w1ab", [128, 2, 128], bf16, isOutput=False)
    w1cb = nc.declare_dram_parameter("w1cb", [128, 2, 128], bf16, isOutput=False)
    w2pb = nc.declare_dram_parameter("w2pb", [128, 2, 63], bf16, isOutput=False)
    outd = nc.declare_dram_parameter("out", [4, 32, 256], f32, isOutput=True)

    with tile.TileContext(nc) as tc:
        with (
            tc.tile_pool(name="const", bufs=1) as constp,
            tc.tile_pool(name="data", bufs=1) as datap,
            tc.tile_pool(name="act", bufs=18) as actp,
            tc.tile_pool(name="ph", bufs=1, space="PSUM") as php,
            tc.tile_pool(name="pc", bufs=2, space="PSUM") as pcp,
            tc.tile_pool(name="pl", bufs=3, space="PSUM") as plp,
        ):
            # ---- DMAs: arrival order matched to the hT octet program order ----
            nf_sb = constp.tile([128, 32, 128], bf16, tag="nf")
            wee_sb = constp.tile([128, _WEE_W], f32, tag="wee")
            w1cb_sb = constp.tile([128, 2, 128], bf16, tag="w1cb")
            w1ab_sb = constp.tile([128, 2, 128], bf16, tag="w1ab")
            w2pb_sb = constp.tile([128, 2, 63], bf16, tag="w2pb")
            nc.sync.dma_start(out=nf_sb[:, 0:8, :], in_=nf[:, 0:8, :])
            nc.scalar.dma_start(out=wee_sb[:], in_=wee[:])
            nc.scalar.dma_start(out=nf_sb[:, 8:16, :], in_=nf[:, 8:16, :])
            nc.sync.dma_start(out=nf_sb[:, 16:20, :], in_=nf[:, 16:20, :])
            nc.scalar.dma_start(out=nf_sb[:, 20:24, :], in_=nf[:, 20:24, :])
            nc.sync.dma_start(out=nf_sb[:, 24:28, :], in_=nf[:, 24:28, :])
            nc.scalar.dma_start(out=nf_sb[:, 28:32, :], in_=nf[:, 28:32, :])
            nc.sync.dma_start(out=w1ab_sb[:], in_=w1ab[:])
            nc.scalar.dma_start(out=w1cb_sb[:], in_=w1cb[:])
            nc.sync.dma_start(out=w2pb_sb[:], in_=w2pb[:])

            # ---- PE warm-up + early SE act-table load during DMA wait ----
            scratch = constp.tile([128, 8], bf16, tag="scratch")
            nc.vector.memset(scratch[:], 0.0)
            scratch2 = constp.tile([128, 8], bf16, tag="scratch2")
            nc.scalar.activation(scratch2[:], scratch[:], Act.Relu)
            warm_rhs = constp.tile([128, 256], bf16, tag="warm_rhs")
            nc.vector.memset(warm_rhs[:], 0.0)
            ph = php.tile([128, 256], f32, tag="ph")

            def warm(n):
                # wide junk matmuls (N=256 keeps the PE array dense enough to
                # trip the HAM un-throttle); ph is fully overwritten by the
                # hT octet matmuls later (same-engine program order)
                for _ in range(n):
                    nc.tensor.matmul(
                        ph[:, 0:256], lhsT=warm_rhs[:, 0:128], rhs=warm_rhs[:],
                        start=True, stop=True,
                    )

            warm(20)

            # smat cast f32 -> bf16
            smat_sb = constp.tile([128, 8], bf16, tag="smat")
            nc.vector.tensor_copy(smat_sb[:], wee_sb[:, _WEE_SMAT : _WEE_SMAT + 8])

            # ---- hT[f, j]: octets 0..15 first, then aT (which needs only
            # the core's own i-half columns) while octets 16..31 arrive ----
            cT = [
                datap.tile([128, 256], bf16, tag=f"cT{t}", name=f"cT{t}")
                for t in range(2)
            ]
            aTb4 = [
                datap.tile([128, 128, 4], f32, tag=f"aTb4{t}", name=f"aTb4{t}")
                for t in range(2)
            ]
            for o in range(16):
                nc.tensor.matmul(
                    ph[:, 8 * o : 8 * o + 8],
                    lhsT=nf_sb[:, o, :],
                    rhs=smat_sb[:],
                    start=True,
                    stop=True,
                )
            hTb = datap.tile([128, 256], bf16, tag="hTb")
            nc.vector.tensor_copy(hTb[:, 0:128], ph[:, 0:128])
            for t in range(2):
                pa = pcp.tile([128, 128], f32, tag="pa", name=f"pa{t}")
                nc.tensor.matmul(
                    pa[:], lhsT=w1ab_sb[:, t, :], rhs=hTb[:, 0:128], start=True, stop=True
                )
                nc.scalar.activation(
                    aTb4[t][:, :, :],
                    pa[:].broadcast_to([128, 128, 4]),
                    Act.Identity,
                    bias=wee_sb[:, _WEE_B1T0 + 4 * t : _WEE_B1T0 + 4 * t + 1],
                )
            for o in range(16, 32):
                nc.tensor.matmul(
                    ph[:, 8 * o : 8 * o + 8],
                    lhsT=nf_sb[:, o, :],
                    rhs=smat_sb[:],
                    start=True,
                    stop=True,
                )
            nc.vector.tensor_copy(hTb[:, 128:256], ph[:, 128:256])
            for t in range(2):
                pc = pcp.tile([128, 256], f32, tag="pc", name=f"pc{t}")
                nc.tensor.matmul(
                    pc[:], lhsT=w1cb_sb[:, t, :], rhs=hTb[:], start=True, stop=True
                )
                if t == 0:
                    nc.scalar.copy(cT[t][:], pc[:])
                else:
                    nc.vector.tensor_copy(cT[t][:], pc[:])

            # ---- pairwise: act tiles on VE/SE, reduction on PE ----
            # stationary for (t, r): w2pb[:, t, 31-r : 63-r]  (col r = w2_t)
            for g in range(4):
                pl = plp.tile([32, 256], f32, tag="pl", name=f"pl{g}")
                for t in range(2):
                    for r in range(32):
                        i = 32 * g + r
                        idx = 2 * i + t
                        a_col = aTb4[t][:, i, 0:1]
                        if idx % 3 == 1 or idx % 64 == 0:
                            asb = actp.tile([128, 256], bf16, tag="acts")
                            nc.scalar.activation(
                                asb[:], cT[t][:], Act.Relu, bias=a_col
                            )
                            mv = asb
                        else:
                            av = actp.tile([128, 256], bf16, tag="actv")
                            nc.vector.tensor_scalar(
                                av[:], cT[t][:], a_col, 0.0, Alu.add, Alu.max
                            )
                            mv = av
                        nc.tensor.matmul(
                            pl[:, :],
                            lhsT=w2pb_sb[:, t, 31 - r : 63 - r],
                            rhs=mv[:],
                            start=(t == 0 and r == 0),
                            stop=(t == 1 and r == 31),
                        )
                osb = datap.tile([32, 256], f32, tag=f"osb{g}", name=f"osb{g}")
                nc.vector.tensor_scalar(
                    osb[:],
                    pl[:, :],
                    wee_sb[0:32, _WEE_B2C : _WEE_B2C + 1],
                    None,
                    Alu.add,
                )
                nc.sync.dma_start(out=outd[g], in_=osb[:])

    nc.compile()
    return nc


def make_in_maps(nodefeat, W1, b1, W2, b2):
    """Host-side sharding/layout prep."""
    import ml_dtypes

    bf16 = ml_dtypes.bfloat16
    nodefeat = np.asarray(nodefeat, dtype=np.float32)
    W1 = np.asarray(W1, dtype=np.float32)
    b1 = np.asarray(b1, dtype=np.float32)
    W2 = np.asarray(W2, dtype=np.float32)
    b2 = np.asarray(b2, dtype=np.float32)

    wee = np.zeros((128, _WEE_W), dtype=np.float32)
    wee[:, _WEE_SMAT : _WEE_SMAT + 8] = (
        np.repeat(np.eye(8, dtype=np.float32), 16, axis=0) / 16.0
    )
    b1t = b1.reshape(2, 128).T  # [p, t]
    wee[:, _WEE_B1T0] = b1t[:, 0]
    wee[:, _WEE_B1T1] = b1t[:, 1]
    wee[:, _WEE_B2C] = b2[0]

    W1a, W1c = W1[:, :_F], W1[:, _F:]
    w1ab = np.ascontiguousarray(
        np.stack([W1a[:128].T, W1a[128:].T], axis=1).astype(bf16)
    )
    w1cb = np.ascontiguousarray(
        np.stack([W1c[:128].T, W1c[128:].T], axis=1).astype(bf16)
    )

    w2r = W2[0].reshape(2, 128)  # [t, p]
    w2pad = np.zeros((128, 2, 63), dtype=np.float32)
    w2pad[:, :, 31] = w2r.T
    w2pb = np.ascontiguousarray(w2pad.astype(bf16))

    in_maps = []
    for k in range(_NCORES):
        b, ih = divmod(k, 2)
        nf_b = nodefeat[b]  # [256, 16, 128]
        if ih:
            nf_b = np.concatenate([nf_b[128:], nf_b[:128]], axis=0)
        # [256,16,128] -> [32 oct, (j8,t16)=128, 128 f] -> [128, 32, 128]
        nf_dev = np.ascontiguousarray(
            nf_b.reshape(32, 128, 128).transpose(1, 0, 2).astype(bf16)
        )
        in_maps.append(
            {"nf": nf_dev, "wee": wee, "w1ab": w1ab, "w1cb": w1cb, "w2pb": w2pb}
        )
    return in_maps


def assemble_output(results):
    out = np.empty((_B, _N, _N), dtype=np.float32)
    for k in range(_NCORES):
        b, ih = divmod(k, 2)
        r = results[k]["out"].reshape(128, 256)  # [i, j] (j core-local order)
        if ih:
            r = np.concatenate([r[:, 128:], r[:, :128]], axis=1)
        out[b, ih * 128 : (ih + 1) * 128, :] = r
    return out


def _get_nc():
    if "nc" not in _CACHE:
        _CACHE["nc"] = build_nc()
    return _CACHE["nc"]


def kernel(nodefeat, W1, b1, W2, b2):
    _ensure_paths()
    from concourse.bass_utils import run_bass_kernel_spmd

    nc = _get_nc()
    in_maps = make_in_maps(nodefeat, W1, b1, W2, b2)
    res = run_bass_kernel_spmd(nc, in_maps, list(range(_NCORES)))
    return assemble_output(res.results)


# revision 24
# speedup vs baseline: 1.0075x; 1.0075x over previous
"""Trainium2 Bass kernel for nn_LinkPredictor.

Reference computation (B=4, N=256, T=16, F=128, H=256):
    h = mean_T(nodefeat)                      # [B,N,F]
    a = h @ W1[:, :F].T                       # [B,N,H]
    c = h @ W1[:, F:].T                       # [B,N,H]
    logits[b,i,j] = W2[0] . relu(a[b,i] + c[b,j] + b1) + b2   # [B,N,N]

Sharding: 8 cores; core k handles batch b=k//2, i-half k%2 (128 i-rows x
256 j-cols of one batch's NxN grid).  Each core only needs nodefeat[b].

v4 = baseline pairwise structure (tight act pool, in-loop drains, VE/SE
act split) + a lean prologue:
  - inputs on BOTH HWDGE queues (sync+scalar), arrival-ordered; 0.68MB
    less traffic than baseline (w2 diagonal matrix -> [128,2,63] sliding
    window; W1 in bf16; smat/b1/b2 packed into one tiny param).
  - wide dummy matmuls during the DMA wait pre-trip the HAM clock gate
    (PE at 2.4GHz when the pairwise phase starts instead of ~10us in).
  - early ScalarE activation-table load.
"""

import os
import sys

import numpy as np

_B, _N, _T, _F, _H = 4, 256, 16, 128, 256
_NCORES = 8

_CACHE = {}


def _ensure_paths():
    for p in (
        "/root/.axon_site",
        "/root/.axon_site/_ro/trn_rl_repo",
        "/root/.axon_site/_ro/pypackages",
        "/opt/trn_rl_repo",
    ):
        if os.path.isdir(p) and p not in sys.path:
            sys.path.append(p)


# wee param layout (f32 columns; scalar cols 16B-aligned)
_WEE_SMAT = 0   # [0:8) smat
_WEE_B1T0 = 8   # b1 (t=0)
_WEE_B1T1 = 12  # b1 (t=1)
_WEE_B2C = 16   # b2
_WEE_W = 20


def build_nc():
    """Build the per-core Bass program (same program for all 8 cores)."""
    _ensure_paths()
    import concourse.mybir as mybir
    import concourse.tile as tile
    from concourse import bacc

    f32 = mybir.dt.float32
    bf16 = mybir.dt.bfloat16
    Alu = mybir.AluOpType
    Act = mybir.ActivationFunctionType

    nc = bacc.Bacc("TRN2", target_bir_lowering=False, debug=False)

    nf = nc.declare_dram_parameter("nf", [128, 32, 128], bf16, isOutput=False)
    wee = nc.declare_dram_parameter("wee", [128, _WEE_W], f32, isOutput=False)
    w1ab = nc.declare_dram_parameter("w1ab", [128, 2, 128], bf16, isOutput=False)
    w1cb = nc.declare_dram_parameter("w1cb", [128, 2, 128], bf16, isOutput=False)
    w2pb = nc.declare_dram_parameter("w2pb", [128, 2, 63], bf16, isOutput=False)
    outd = nc.declare_dram_parameter("out", [4, 32, 256], f32, isOutput=True)

    with tile.TileContext(nc) as tc:
        with (
            tc.tile_pool(name="const", bufs=1) as constp,
            tc.tile_pool(name="data", bufs=1) as datap,
            tc.tile_pool(name="act", bufs=18) as actp,
            tc.tile_pool(name="ph", bufs=1, space="PSUM") as php,
            tc.tile_pool(name="pc", bufs=2, space="PSUM") as pcp,
            tc.tile_pool(name="pl", bufs=3, space="PSUM") as plp,
        ):
            # ---- DMAs: arrival order matched to the hT octet program order ----
            nf_sb = constp.tile([128, 32, 128], bf16, tag="nf")
            wee_sb = constp.tile([128, _WEE_W], f32, tag="wee")
            w1cb_sb = constp.tile([128, 2, 128], bf16, tag="w1cb")
            w1ab_sb = constp.tile([128, 2, 128], bf16, tag="w1ab")
            w2pb_sb = constp.tile([128, 2, 63], bf16, tag="w2pb")
            nc.sync.dma_start(out=nf_sb[:, 0:8, :], in_=nf[:, 0:8, :])
            nc.scalar.dma_start(out=wee_sb[:], in_=wee[:])
            nc.scalar.dma_start(out=nf_sb[:, 8:16, :], in_=nf[:, 8:16, :])
            nc.sync.dma_start(out=nf_sb[:, 16:20, :], in_=nf[:, 16:20, :])
            nc.scalar.dma_start(out=nf_sb[:, 20:24, :], in_=nf[:, 20:24, :])
            nc.sync.dma_start(out=nf_sb[:, 24:28, :], in_=nf[:, 24:28, :])
            nc.scalar.dma_start(out=nf_sb[:, 28:32, :], in_=nf[:, 28:32, :])
            nc.sync.dma_start(out=w1ab_sb[:], in_=w1ab[:])
            nc.scalar.dma_start(out=w1cb_sb[:], in_=w1cb[:])
            nc.sync.dma_start(out=w2pb_sb[:], in_=w2pb[:])

            # ---- PE warm-up + early SE act-table load during DMA wait ----
            scratch = constp.tile([128, 8], bf16, tag="scratch")
            nc.vector.memset(scratch[:], 0.0)
            scratch2 = constp.tile([128, 8], bf16, tag="scratch2")
            nc.scalar.activation(scratch2[:], scratch[:], Act.Relu)
            warm_rhs = constp.tile([128, 256], bf16, tag="warm_rhs")
            nc.vector.memset(warm_rhs[:], 0.0)
            ph = php.tile([128, 256], f32, tag="ph")

            def warm(n):
                # wide junk matmuls (N=256 keeps the PE array dense enough to
                # trip the HAM un-throttle); ph is fully overwritten by the
                # hT octet matmuls later (same-engine program order)
                for _ in range(n):
                    nc.tensor.matmul(
                        ph[:, 0:256], lhsT=warm_rhs[:, 0:128], rhs=warm_rhs[:],
                        start=True, stop=True,
                    )

            warm(24)

            # smat cast f32 -> bf16
            smat_sb = constp.tile([128, 8], bf16, tag="smat")
            nc.vector.tensor_copy(smat_sb[:], wee_sb[:, _WEE_SMAT : _WEE_SMAT + 8])

            # ---- hT[f, j]: octets 0..15 first, then aT (which needs only
            # the core's own i-half columns) while octets 16..31 arrive ----
            cT = [
                datap.tile([128, 256], bf16, tag=f"cT{t}", name=f"cT{t}")
                for t in range(2)
            ]
            aTb4 = [
                datap.tile([128, 128, 4], f32, tag=f"aTb4{t}", name=f"aTb4{t}")
                for t in range(2)
            ]
            for o in range(16):
                nc.tensor.matmul(
                    ph[:, 8 * o : 8 * o + 8],
                    lhsT=nf_sb[:, o, :],
                    rhs=smat_sb[:],
                    start=True,
                    stop=True,
                )
            hTb = datap.tile([128, 256], bf16, tag="hTb")
            nc.vector.tensor_copy(hTb[:, 0:128], ph[:, 0:128])
            for t in range(2):
                pa = pcp.tile([128, 128], f32, tag="pa", name=f"pa{t}")
                nc.tensor.matmul(
                    pa[:], lhsT=w1ab_sb[:, t, :], rhs=hTb[:, 0:128], start=True, stop=True
                )
                nc.scalar.activation(
                    aTb4[t][:, :, :],
                    pa[:].broadcast_to([128, 128, 4]),
                    Act.Identity,
                    bias=wee_sb[:, _WEE_B1T0 + 4 * t : _WEE_B1T0 + 4 * t + 1],
                )
            for o in range(16, 32):
                nc.tensor.matmul(
                    ph[:, 8 * o : 8 * o + 8],
                    lhsT=nf_sb[:, o, :],
                    rhs=smat_sb[:],
                    start=True,
                    stop=True,
                )
            nc.vector.tensor_copy(hTb[:, 128:256], ph[:, 128:256])
            for t in range(2):
                pc = pcp.tile([128, 256], f32, tag="pc", name=f"pc{t}")
                nc.tensor.matmul(
                    pc[:], lhsT=w1cb_sb[:, t, :], rhs=hTb[:], start=True, stop=True
                )
                if t == 0:
                    nc.scalar.copy(cT[t][:], pc[:])
                else:
                    nc.vector.tensor_copy(cT[t][:], pc[:])

            # ---- pairwise: act tiles on VE/SE, reduction on PE ----
            # stationary for (t, r): w2pb[:, t, 31-r : 63-r]  (col r = w2_t)
            for g in range(4):
                pl = plp.tile([32, 256], f32, tag="pl", name=f"pl{g}")
                for t in range(2):
                    for r in range(32):
                        i = 32 * g + r
                        idx = 2 * i + t
                        a_col = aTb4[t][:, i, 0:1]
                        if idx % 3 == 1 or idx % 64 == 0:
                            asb = actp.tile([128, 256], bf16, tag="acts")
                            nc.scalar.activation(
                                asb[:], cT[t][:], Act.Relu, bias=a_col
                            )
                            mv = asb
                        else:
                            av = actp.tile([128, 256], bf16, tag="actv")
                            nc.vector.tensor_scalar(
                                av[:], cT[t][:], a_col, 0.0, Alu.add, Alu.max
                            )
                            mv = av
                        nc.tensor.matmul(
                            pl[:, :],
                            lhsT=w2pb_sb[:, t, 31 - r : 63 - r],
                            rhs=mv[:],
                            start=(t == 0 and r == 0),
                            stop=(t == 1 and r == 31),
                        )
                osb = datap.tile([32, 256], f32, tag=f"osb{g}", name=f"osb{g}")
                nc.vector.tensor_scalar(
                    osb[:],
                    pl[:, :],
                    wee_sb[0:32, _WEE_B2C : _WEE_B2C + 1],
                    None,
                    Alu.add,
                )
                nc.sync.dma_start(out=outd[g], in_=osb[:])

    nc.compile()
    return nc


def make_in_maps(nodefeat, W1, b1, W2, b2):
    """Host-side sharding/layout prep."""
    import ml_dtypes

    bf16 = ml_dtypes.bfloat16
    nodefeat = np.asarray(nodefeat, dtype=np.float32)
    W1 = np.asarray(W1, dtype=np.float32)
    b1 = np.asarray(b1, dtype=np.float32)
    W2 = np.asarray(W2, dtype=np.float32)
    b2 = np.asarray(b2, dtype=np.float32)

    wee = np.zeros((128, _WEE_W), dtype=np.float32)
    wee[:, _WEE_SMAT : _WEE_SMAT + 8] = (
        np.repeat(np.eye(8, dtype=np.float32), 16, axis=0) / 16.0
    )
    b1t = b1.reshape(2, 128).T  # [p, t]
    wee[:, _WEE_B1T0] = b1t[:, 0]
    wee[:, _WEE_B1T1] = b1t[:, 1]
    wee[:, _WEE_B2C] = b2[0]

    W1a, W1c = W1[:, :_F], W1[:, _F:]
    w1ab = np.ascontiguousarray(
        np.stack([W1a[:128].T, W1a[128:].T], axis=1).astype(bf16)
    )
    w1cb = np.ascontiguousarray(
        np.stack([W1c[:128].T, W1c[128:].T], axis=1).astype(bf16)
    )

    w2r = W2[0].reshape(2, 128)  # [t, p]
    w2pad = np.zeros((128, 2, 63), dtype=np.float32)
    w2pad[:, :, 31] = w2r.T
    w2pb = np.ascontiguousarray(w2pad.astype(bf16))

    in_maps = []
    for k in range(_NCORES):
        b, ih = divmod(k, 2)
        nf_b = nodefeat[b]  # [256, 16, 128]
        if ih:
            nf_b = np.concatenate([nf_b[128:], nf_b[:128]], axis=0)
        # [256,16,128] -> [32 oct, (j8,t16)=128, 128 f] -> [128, 32, 128]
        nf_dev = np.ascontiguousarray(
            nf_b.reshape(32, 128, 128).transpose(1, 0, 2).astype(bf16)
        )
        in_maps.append(
            {"nf": nf_dev, "wee": wee, "w1ab": w1ab, "w1cb": w1cb, "w2pb": w2pb}
        )
    return in_maps


def assemble_output(results):
    out = np.empty((_B, _N, _N), dtype=np.float32)
    for k in range(_NCORES):
        b, ih = divmod(k, 2)
        r = results[k]["out"].reshape(128, 256)  # [i, j] (j core-local order)
        if ih:
            r = np.concatenate([r[:, 128:], r[:, :128]], axis=1)
        out[b, ih * 128 : (ih + 1) * 128, :] = r
    return out


def _get_nc():
    if "nc" not in _CACHE:
        _CACHE["nc"] = build_nc()
    return _CACHE["nc"]


def kernel(nodefeat, W1, b1, W2, b2):
    _ensure_paths()
    from concourse.bass_utils import run_bass_kernel_spmd

    nc = _get_nc()
    in_maps = make_in_maps(nodefeat, W1, b1, W2, b2)
    res = run_bass_kernel_spmd(nc, in_maps, list(range(_NCORES)))
    return assemble_output(res.results)


# revision 25
# speedup vs baseline: 1.0128x; 1.0053x over previous
"""Trainium2 Bass kernel for nn_LinkPredictor.

Reference computation (B=4, N=256, T=16, F=128, H=256):
    h = mean_T(nodefeat)                      # [B,N,F]
    a = h @ W1[:, :F].T                       # [B,N,H]
    c = h @ W1[:, F:].T                       # [B,N,H]
    logits[b,i,j] = W2[0] . relu(a[b,i] + c[b,j] + b1) + b2   # [B,N,N]

Sharding: 8 cores; core k handles batch b=k//2, i-half k%2 (128 i-rows x
256 j-cols of one batch's NxN grid).  Each core only needs nodefeat[b].

v4 = baseline pairwise structure (tight act pool, in-loop drains, VE/SE
act split) + a lean prologue:
  - inputs on BOTH HWDGE queues (sync+scalar), arrival-ordered; 0.68MB
    less traffic than baseline (w2 diagonal matrix -> [128,2,63] sliding
    window; W1 in bf16; smat/b1/b2 packed into one tiny param).
  - wide dummy matmuls during the DMA wait pre-trip the HAM clock gate
    (PE at 2.4GHz when the pairwise phase starts instead of ~10us in).
  - early ScalarE activation-table load.
"""

import os
import sys

import numpy as np

_B, _N, _T, _F, _H = 4, 256, 16, 128, 256
_NCORES = 8

_CACHE = {}


def _ensure_paths():
    for p in (
        "/root/.axon_site",
        "/root/.axon_site/_ro/trn_rl_repo",
        "/root/.axon_site/_ro/pypackages",
        "/opt/trn_rl_repo",
    ):
        if os.path.isdir(p) and p not in sys.path:
            sys.path.append(p)


# wee param layout (f32 columns; scalar cols 16B-aligned)
_WEE_SMAT = 0   # [0:8) smat
_WEE_B1T0 = 8   # b1 (t=0)
_WEE_B1T1 = 12  # b1 (t=1)
_WEE_B2C = 16   # b2
_WEE_B2ROW = 20  # [20:52) = b2 on partition 0 (lhsT of the b2-add matmul)
_WEE_W = 52


def build_nc():
    """Build the per-core Bass program (same program for all 8 cores)."""
    _ensure_paths()
    import concourse.mybir as mybir
    import concourse.tile as tile
    from concourse import bacc

    f32 = mybir.dt.float32
    bf16 = mybir.dt.bfloat16
    Alu = mybir.AluOpType
    Act = mybir.ActivationFunctionType

    nc = bacc.Bacc("TRN2", target_bir_lowering=False, debug=False)

    nf = nc.declare_dram_parameter("nf", [128, 32, 128], bf16, isOutput=False)
    wee = nc.declare_dram_parameter("wee", [128, _WEE_W], f32, isOutput=False)
    w1ab = nc.declare_dram_parameter("w1ab", [128, 2, 128], bf16, isOutput=False)
    w1cb = nc.declare_dram_parameter("w1cb", [128, 2, 128], bf16, isOutput=False)
    w2pb = nc.declare_dram_parameter("w2pb", [128, 2, 63], bf16, isOutput=False)
    outd = nc.declare_dram_parameter("out", [4, 32, 256], f32, isOutput=True)

    with tile.TileContext(nc) as tc:
        with (
            tc.tile_pool(name="const", bufs=1) as constp,
            tc.tile_pool(name="data", bufs=1) as datap,
            tc.tile_pool(name="act", bufs=18) as actp,
            tc.tile_pool(name="ph", bufs=1, space="PSUM") as php,
            tc.tile_pool(name="pc", bufs=2, space="PSUM") as pcp,
            tc.tile_pool(name="pl", bufs=3, space="PSUM") as plp,
        ):
            # ---- DMAs: arrival order matched to the hT octet program order ----
            nf_sb = constp.tile([128, 32, 128], bf16, tag="nf")
            wee_sb = constp.tile([128, _WEE_W], f32, tag="wee")
            w1cb_sb = constp.tile([128, 2, 128], bf16, tag="w1cb")
            w1ab_sb = constp.tile([128, 2, 128], bf16, tag="w1ab")
            w2pb_sb = constp.tile([128, 2, 63], bf16, tag="w2pb")
            nc.sync.dma_start(out=nf_sb[:, 0:8, :], in_=nf[:, 0:8, :])
            nc.scalar.dma_start(out=wee_sb[:], in_=wee[:])
            nc.scalar.dma_start(out=nf_sb[:, 8:16, :], in_=nf[:, 8:16, :])
            nc.sync.dma_start(out=nf_sb[:, 16:20, :], in_=nf[:, 16:20, :])
            nc.scalar.dma_start(out=nf_sb[:, 20:24, :], in_=nf[:, 20:24, :])
            nc.sync.dma_start(out=nf_sb[:, 24:28, :], in_=nf[:, 24:28, :])
            nc.scalar.dma_start(out=nf_sb[:, 28:32, :], in_=nf[:, 28:32, :])
            nc.sync.dma_start(out=w1ab_sb[:], in_=w1ab[:])
            nc.scalar.dma_start(out=w1cb_sb[:], in_=w1cb[:])
            nc.sync.dma_start(out=w2pb_sb[:], in_=w2pb[:])

            # ---- PE warm-up + early SE act-table load during DMA wait ----
            scratch = constp.tile([128, 8], bf16, tag="scratch")
            nc.vector.memset(scratch[:], 0.0)
            scratch2 = constp.tile([128, 8], bf16, tag="scratch2")
            nc.scalar.activation(scratch2[:], scratch[:], Act.Relu)
            warm_rhs = constp.tile([128, 256], bf16, tag="warm_rhs")
            nc.vector.memset(warm_rhs[:], 0.0)
            ph = php.tile([128, 256], f32, tag="ph")

            def warm(n):
                # wide junk matmuls (N=256 keeps the PE array dense enough to
                # trip the HAM un-throttle); ph is fully overwritten by the
                # hT octet matmuls later (same-engine program order)
                for _ in range(n):
                    nc.tensor.matmul(
                        ph[:, 0:256], lhsT=warm_rhs[:, 0:128], rhs=warm_rhs[:],
                        start=True, stop=True,
                    )

            warm(24)

            # smat cast f32 -> bf16
            smat_sb = constp.tile([128, 8], bf16, tag="smat")
            nc.vector.tensor_copy(smat_sb[:], wee_sb[:, _WEE_SMAT : _WEE_SMAT + 8])
            ones_sb = constp.tile([128, 256], bf16, tag="ones")
            nc.vector.memset(ones_sb[:], 1.0)
            b2r_sb = constp.tile([128, 32], bf16, tag="b2r")
            nc.vector.tensor_copy(
                b2r_sb[0:1, :], wee_sb[0:1, _WEE_B2ROW : _WEE_B2ROW + 32]
            )

            # ---- hT[f, j]: octets 0..15 first, then aT (which needs only
            # the core's own i-half columns) while octets 16..31 arrive ----
            cT = [
                datap.tile([128, 256], bf16, tag=f"cT{t}", name=f"cT{t}")
                for t in range(2)
            ]
            aTb4 = [
                datap.tile([128, 128, 4], f32, tag=f"aTb4{t}", name=f"aTb4{t}")
                for t in range(2)
            ]
            for o in range(16):
                nc.tensor.matmul(
                    ph[:, 8 * o : 8 * o + 8],
                    lhsT=nf_sb[:, o, :],
                    rhs=smat_sb[:],
                    start=True,
                    stop=True,
                )
            hTb = datap.tile([128, 256], bf16, tag="hTb")
            nc.vector.tensor_copy(hTb[:, 0:128], ph[:, 0:128])
            for t in range(2):
                pa = pcp.tile([128, 128], f32, tag="pa", name=f"pa{t}")
                nc.tensor.matmul(
                    pa[:], lhsT=w1ab_sb[:, t, :], rhs=hTb[:, 0:128], start=True, stop=True
                )
                nc.scalar.activation(
                    aTb4[t][:, :, :],
                    pa[:].broadcast_to([128, 128, 4]),
                    Act.Identity,
                    bias=wee_sb[:, _WEE_B1T0 + 4 * t : _WEE_B1T0 + 4 * t + 1],
                )
            for o in range(16, 32):
                nc.tensor.matmul(
                    ph[:, 8 * o : 8 * o + 8],
                    lhsT=nf_sb[:, o, :],
                    rhs=smat_sb[:],
                    start=True,
                    stop=True,
                )
            nc.vector.tensor_copy(hTb[:, 128:256], ph[:, 128:256])
            for t in range(2):
                pc = pcp.tile([128, 256], f32, tag="pc", name=f"pc{t}")
                nc.tensor.matmul(
                    pc[:], lhsT=w1cb_sb[:, t, :], rhs=hTb[:], start=True, stop=True
                )
                if t == 0:
                    nc.scalar.copy(cT[t][:], pc[:])
                else:
                    nc.vector.tensor_copy(cT[t][:], pc[:])

            # ---- pairwise: act tiles on VE/SE, reduction on PE ----
            # stationary for (t, r): w2pb[:, t, 31-r : 63-r]  (col r = w2_t)
            for g in range(4):
                pl = plp.tile([32, 256], f32, tag="pl", name=f"pl{g}")
                for t in range(2):
                    for r in range(32):
                        i = 32 * g + r
                        idx = 2 * i + t
                        a_col = aTb4[t][:, i, 0:1]
                        if idx % 3 == 1 or idx % 64 == 0:
                            asb = actp.tile([128, 256], bf16, tag="acts")
                            nc.scalar.activation(
                                asb[:], cT[t][:], Act.Relu, bias=a_col
                            )
                            mv = asb
                        else:
                            av = actp.tile([128, 256], bf16, tag="actv")
                            nc.vector.tensor_scalar(
                                av[:], cT[t][:], a_col, 0.0, Alu.add, Alu.max
                            )
                            mv = av
                        nc.tensor.matmul(
                            pl[:, :],
                            lhsT=w2pb_sb[:, t, 31 - r : 63 - r],
                            rhs=mv[:],
                            start=(t == 0 and r == 0),
                            stop=False,
                        )
                # +b2 via a 1-partition matmul closing the group (start=False,
                # so it cannot serialize on the previous group's drain)
                nc.tensor.matmul(
                    pl[:, :],
                    lhsT=b2r_sb[0:1, :],
                    rhs=ones_sb[0:1, :],
                    start=False,
                    stop=True,
                )
                osb = datap.tile([32, 256], f32, tag=f"osb{g}", name=f"osb{g}")
                nc.vector.tensor_copy(osb[:], pl[:, :])
                nc.sync.dma_start(out=outd[g], in_=osb[:])

    nc.compile()
    return nc


def make_in_maps(nodefeat, W1, b1, W2, b2):
    """Host-side sharding/layout prep."""
    import ml_dtypes

    bf16 = ml_dtypes.bfloat16
    nodefeat = np.asarray(nodefeat, dtype=np.float32)
    W1 = np.asarray(W1, dtype=np.float32)
    b1 = np.asarray(b1, dtype=np.float32)
    W2 = np.asarray(W2, dtype=np.float32)
    b2 = np.asarray(b2, dtype=np.float32)

    wee = np.zeros((128, _WEE_W), dtype=np.float32)
    wee[:, _WEE_SMAT : _WEE_SMAT + 8] = (
        np.repeat(np.eye(8, dtype=np.float32), 16, axis=0) / 16.0
    )
    b1t = b1.reshape(2, 128).T  # [p, t]
    wee[:, _WEE_B1T0] = b1t[:, 0]
    wee[:, _WEE_B1T1] = b1t[:, 1]
    wee[:, _WEE_B2C] = b2[0]
    wee[0, _WEE_B2ROW : _WEE_B2ROW + 32] = b2[0]

    W1a, W1c = W1[:, :_F], W1[:, _F:]
    w1ab = np.ascontiguousarray(
        np.stack([W1a[:128].T, W1a[128:].T], axis=1).astype(bf16)
    )
    w1cb = np.ascontiguousarray(
        np.stack([W1c[:128].T, W1c[128:].T], axis=1).astype(bf16)
    )

    w2r = W2[0].reshape(2, 128)  # [t, p]
    w2pad = np.zeros((128, 2, 63), dtype=np.float32)
    w2pad[:, :, 31] = w2r.T
    w2pb = np.ascontiguousarray(w2pad.astype(bf16))

    in_maps = []
    for k in range(_NCORES):
        b, ih = divmod(k, 2)
        nf_b = nodefeat[b]  # [256, 16, 128]
        if ih:
            nf_b = np.concatenate([nf_b[128:], nf_b[:128]], axis=0)
        # [256,16,128] -> [32 oct, (j8,t16)=128, 128 f] -> [128, 32, 128]
        nf_dev = np.ascontiguousarray(
            nf_b.reshape(32, 128, 128).transpose(1, 0, 2).astype(bf16)
        )
        in_maps.append(
            {"nf": nf_dev, "wee": wee, "w1ab": w1ab, "w1cb": w1cb, "w2pb": w2pb}
        )
    return in_maps


def assemble_output(results):
    out = np.empty((_B, _N, _N), dtype=np.float32)
    for k in range(_NCORES):
        b, ih = divmod(k, 2)
        r = results[k]["out"].reshape(128, 256)  # [i, j] (j core-local order)
        if ih:
            r = np.concatenate([r[:, 128:], r[:, :128]], axis=1)
        out[b, ih * 128 : (ih + 1) * 128, :] = r
    return out


def _get_nc():
    if "nc" not in _CACHE:
        _CACHE["nc"] = build_nc()
    return _CACHE["nc"]


def kernel(nodefeat, W1, b1, W2, b2):
    _ensure_paths()
    from concourse.bass_utils import run_bass_kernel_spmd

    nc = _get_nc()
    in_maps = make_in_maps(nodefeat, W1, b1, W2, b2)
    res = run_bass_kernel_spmd(nc, in_maps, list(range(_NCORES)))
    return assemble_output(res.results)
